# revision 3
# baseline (speedup 1.0000x reference)
"""ArcFace loss on 8 TRN2 NeuronCores — class-parallel, fp8, hybrid exp.

Math: loss = mean_b[ M0 + ln(Z'_b) - s*phi_b ] with
  Z_b  = sum_c exp(s*cos(b,c) - M0)          (device, sharded over classes)
  Z'_b = Z_b - dev(b, l_b) + exp(s*phi_b - M0)
dev(b, l) is the device's own contribution for the label class (host
replicates it bit-for-bit so the correction cancels); phi uses the exact
f64 cosine. M0 is a fixed logsumexp shift.

Device per core: x and W rows unit-normalized on host, scaled by QS=32,
fp8 e4m3. The 512x512x12544 matmul runs in DoubleRow perf mode. Loop
order keeps each stationary x-tile loaded for 4 consecutive N=512
matmuls (1 LDWEIGHTS per 4 MMs instead of 1:1). All W superblocks are
DMA'd up front and stay SBUF-resident; ~10 warm-up matmuls on a zeroed
scratch run during the DMA head so the PE HAM clock-gate is released
(2.4 GHz) before real work arrives.

exp is split across two engines per 2048-col superblock:
 - ACT: cols [0:GA): exp activation with accum_out (sum comes for free,
   no DVE fold needed).
 - DVE: cols [GA:2048): Schraudolph bit-trick exp2 — one tensor_scalar
   (p*A + B -> uint32, negative inputs saturate to 0 = free relu; RNE
   convert materializes the f32 bit pattern of 2^t) and one accumulating
   tensor_scalar over the bitcast-f32 values. c = 1.5 - 1/ln2 makes the
   value-weighted mean of the piecewise-linear error exactly 1, so no
   host-side correction factor is needed.
Per-core output is [128, 8]: per batch-tile ACT and DVE partial sums.
The cross-core sum, label correction, ln and mean run on the host in
f64 — no device collective.
"""

import math

import numpy as np

from concourse import bacc, mybir
from concourse.bass_utils import run_bass_kernel_spmd
from concourse.tile import TileContext

NCORES = 8
B = 512
D = 512
C = 100000
CS = 12544  # per-core classes, padded: 8 * 12544 = 100352 >= C
S = 120.0
MARGIN = 0.3
COS_M = math.cos(MARGIN)
SIN_M = math.sin(MARGIN)
TH = math.cos(math.pi - MARGIN)
MM = math.sin(math.pi - MARGIN) * MARGIN
M0 = 40.0  # logsumexp shift
QS = 32.0  # fp8 quantization scale for x and W (unit rows -> |elem*QS| <= 32)
SUPER = 2048  # class columns per superblock
NBLK = 512  # class columns per matmul (one PSUM bank)
SBS = [SUPER] * 6 + [256]  # superblock widths; sum == CS
assert sum(SBS) == CS
GA = 1408  # ACT-exp columns per 2048 superblock (DVE gets SUPER-GA)
GD = SUPER - GA
WARM = 10  # PE warm-up matmuls during the DMA head

LOG2E = 1.4426950408889634
C_SCH = 1.5 - 1.0 / math.log(2.0)  # value-weighted-unbiased Schraudolph shift
A_SCH = float(np.float32(S * LOG2E / (QS * QS) * 8388608.0))
B_SCH = float(np.float32((127.0 - C_SCH - M0 * LOG2E) * 8388608.0))

F32 = mybir.dt.float32
U32 = mybir.dt.uint32
BF16 = mybir.dt.bfloat16
F8 = mybir.dt.float8e4
FN = mybir.ActivationFunctionType
DR = mybir.MatmulPerfMode.DoubleRow
MULT = mybir.AluOpType.mult
ADD = mybir.AluOpType.add

_GRAPH = None
LAST_RESULT = None  # BassKernelResults of the most recent run (for test harness)


def _build_nc(repeat=1):
    """Build the SPMD graph. repeat>1 unrolls the whole body N times into one
    NEFF (timing only: amortizes the per-execute dispatch overhead)."""
    nc = bacc.Bacc("TRN2", target_bir_lowering=False)

    # const AP for the Exp bias (only 0.0/1.0 are pre-registered)
    _cb = nc.alloc_sbuf_tensor(f"const-float32-{-M0}", [128, 1], F32)
    nc.gpsimd.memset(_cb.ap(), -M0)
    nc.const_aps.aps[(F32, -M0)] = _cb.ap()
    nc.all_engine_barrier()

    # x^T fp8, DoubleRow pairs: row kp*128+p, col i*B+b = x[b, (2kp+i)*128+p]
    xt = nc.declare_dram_parameter("xt", [256, 2 * B], F8, isOutput=False)
    # W^T fp8, DoubleRow pairs, superblock-major: per pair row-block and
    # superblock (c0, sw), cols [2*c0 : 2*c0+2*sw] hold [2, sw] row-major
    wt = nc.declare_dram_parameter("wt", [256, 2 * CS], F8, isOutput=False)
    # per-core partials: col 2*bi = ACT sum, 2*bi+1 = DVE sum, row p =
    # batch bi*128+p
    out = nc.declare_dram_parameter("out", [128, 8], F32, isOutput=True)

    with TileContext(nc, num_cores=NCORES) as tc:
        with (
            tc.tile_pool(name="xpool", bufs=1) as xpool,
            tc.tile_pool(name="wpool", bufs=1) as wpool,
            tc.tile_pool(name="spool", bufs=1) as spool,
            tc.tile_pool(name="zpool", bufs=1) as zpool,
            tc.tile_pool(name="psum", bufs=2, space="PSUM") as pp,
        ):
            # PE warm-up: scratch (zeroed) stationary+moving, results
            # discarded. Runs while the x/W DMAs are in flight so HAM
            # reaches K=8/8 before the first real matmul.
            wsc = xpool.tile([128, 2, NBLK], F8, tag="wsc", name="wsc")
            nc.gpsimd.memset(wsc[:], 0.0)
            ps_w = pp.tile([128, SUPER], F32, tag="ps", name="ps_warm")
            for i in range(WARM):
                nc.tensor.matmul(
                    ps_w[:, :NBLK],
                    wsc[:, :, :128],
                    wsc[:],
                    start=True,
                    stop=True,
                    perf_mode=DR,
                )

            # x^T fp8 pair tiles [K=128, sub=2, B]
            xts = []
            for kp in range(2):
                t = xpool.tile([128, 2, B], F8, tag=f"xt{kp}", name=f"xts{kp}")
                nc.sync.dma_start(
                    t[:],
                    xt[kp * 128 : (kp + 1) * 128, :].rearrange(
                        "p (s b) -> p s b", s=2
                    ),
                )
                xts.append(t)

            for rep in range(repeat):
                _body(nc, tc, rep, wpool, spool, zpool, pp, wt, out, xts)

    if not nc.is_finalized():
        nc.finalize()
    return nc


def _body(nc, tc, rep, wpool, spool, zpool, pp, wt, out, xts):
    # all W superblocks up front, SBUF-resident for the whole rep
    wts = []
    c0 = 0
    for sbi, sw in enumerate(SBS):
        pair = []
        for kp in range(2):
            t = wpool.tile(
                [128, 2, sw], F8, tag=f"w{sbi}_{kp}", name=f"w{sbi}_{kp}_{rep}"
            )
            nc.sync.dma_start(
                t[:],
                wt[
                    kp * 128 : (kp + 1) * 128, 2 * c0 : 2 * c0 + 2 * sw
                ].rearrange("p (s c) -> p s c", s=2),
            )
            pair.append(t)
        wts.append(pair)
        c0 += sw

    # per-batch-tile partial sums, one col per superblock (ACT / DVE split)
    zbA = [
        zpool.tile([128, 8], F32, tag=f"za{bi}", name=f"za{bi}_{rep}")
        for bi in range(4)
    ]
    zbD = [
        zpool.tile([128, 8], F32, tag=f"zd{bi}", name=f"zd{bi}_{rep}")
        for bi in range(4)
    ]
    # scratch outputs (values unused; accum_out carries the result)
    exs = spool.tile([128, GA], BF16, tag="exs", name=f"exs_{rep}")
    sus = spool.tile([128, GD], U32, tag="sus", name=f"sus_{rep}")
    su2 = spool.tile([128, GD], BF16, tag="su2", name=f"su2_{rep}")

    for sbi, sw in enumerate(SBS):
        ga = min(GA, sw)
        for bi in range(4):
            ps = pp.tile([128, SUPER], F32, tag="ps", name=f"ps_{rep}")
            for kp in range(2):
                for nb0 in range(0, sw, NBLK):
                    nb = min(NBLK, sw - nb0)
                    nc.tensor.matmul(
                        ps[:, nb0 : nb0 + nb],
                        xts[kp][:, :, bi * 128 : (bi + 1) * 128],
                        wts[sbi][kp][:, :, nb0 : nb0 + nb],
                        start=(kp == 0),
                        stop=(kp == 1),
                        perf_mode=DR,
                    )
            nc.scalar.activation(
                exs[:, :ga],
                ps[:, :ga],
                FN.Exp,
                bias=-M0,
                scale=S / (QS * QS),
                accum_out=zbA[bi][:, sbi : sbi + 1],
            )
            if sw > ga:
                gd = sw - ga
                nc.vector.tensor_scalar(
                    sus[:, :gd], ps[:, ga:sw], A_SCH, B_SCH, MULT, ADD
                )
                nc.vector.tensor_scalar(
                    su2[:, :gd],
                    sus[:, :gd].bitcast(F32),
                    1.0,
                    0.0,
                    MULT,
                    ADD,
                    accum_out=zbD[bi][:, sbi : sbi + 1],
                )

    # fold per-superblock partials -> out[p, 2*bi] (ACT), out[p, 2*bi+1] (DVE)
    zs_all = zpool.tile([128, 8], F32, tag="zsall", name=f"zsall_{rep}")
    for bi in range(4):
        nc.vector.reduce_sum(
            zs_all[:, 2 * bi : 2 * bi + 1],
            zbA[bi][:, : len(SBS)],
            axis=mybir.AxisListType.X,
        )
        nc.vector.reduce_sum(
            zs_all[:, 2 * bi + 1 : 2 * bi + 2],
            zbD[bi][:, : len(SBS) - 1],
            axis=mybir.AxisListType.X,
        )
    nc.sync.dma_start(out[:], zs_all[:])


def _dr_pack(aT):
    """[D, N] (D=512) -> [256, 2*N]: DoubleRow pair layout. Row kp*128+p,
    col i*N+n = aT[(2*kp+i)*128 + p, n]."""
    d, n = aT.shape
    chunks = aT.reshape(4, 128, n)
    pairs = [
        np.stack([chunks[2 * kp], chunks[2 * kp + 1]], axis=1).reshape(
            128, 2 * n
        )
        for kp in range(2)
    ]
    return np.concatenate(pairs, axis=0)


def _host_prep(input, label, weight):
    x = np.asarray(input, dtype=np.float32)
    lab = np.asarray(label).astype(np.int64).ravel()
    w = np.asarray(weight, dtype=np.float32)
    f8 = mybir.dt.np(F8)

    xn64 = x.astype(np.float64)
    xn64 /= np.maximum(
        np.sqrt(np.einsum("bd,bd->b", xn64, xn64))[:, None], 1e-12
    )
    xq = (xn64 * QS).astype(np.float32).astype(f8)  # [B, D] fp8
    xt = np.ascontiguousarray(_dr_pack(xq.astype(np.float32).T).astype(f8))

    wn_inv = 1.0 / np.maximum(
        np.sqrt(np.einsum("cd,cd->c", w, w, dtype=np.float64)), 1e-12
    )
    wn = w * wn_inv[:, None].astype(np.float32)  # [C, D] normalized rows, f32
    wq = (wn * QS).astype(f8)  # [C, D] fp8

    # label terms (tiny): phi from the exact f64 cosine, the Z-correction
    # from the fp8 cosine the device actually summed
    wl = wn[lab].astype(np.float64)  # [B, D]
    cosl = np.einsum("bd,bd->b", xn64, wl)
    cosl = np.clip(cosl, -1.0, 1.0)
    sine = np.sqrt(np.maximum(1.0 - cosl * cosl, 0.0))
    phi = cosl * COS_M - sine * SIN_M
    phi = np.where(cosl > TH, phi, cosl - MM)
    psl = np.einsum(
        "bd,bd->b",
        xq.astype(np.float32),
        wq[lab].astype(np.float32),
        dtype=np.float64,
    )  # device psum value for the label column (= cosq * QS^2)
    post = {"phi": phi, "psl": psl, "lab": lab}

    # class-sharded, transposed, DoubleRow-packed, superblock-major W
    shards = []
    for i in range(NCORES):
        lo, hi = i * CS, min((i + 1) * CS, C)
        sh = np.zeros((CS, D), dtype=f8)
        sh[: hi - lo] = wq[lo:hi]
        packed = _dr_pack(sh.astype(np.float32).T)  # [256, 2*CS], pair layout
        # rearrange cols to superblock-major [2, sw] blocks
        dst = np.empty_like(packed)
        q = 0
        c0 = 0
        for sw in SBS:
            blk = packed.reshape(256, 2, CS)[:, :, c0 : c0 + sw]
            dst[:, q : q + 2 * sw] = blk.reshape(256, 2 * sw)
            q += 2 * sw
            c0 += sw
        shards.append(np.ascontiguousarray(dst.astype(f8)))
    return xt, shards, post


def _schraudolph_host(p):
    """Replicate the device DVE bit-trick for a scalar psum value p (f64)."""
    v = np.float32(np.float32(np.float32(p) * np.float32(A_SCH)) +
                   np.float32(B_SCH))
    if v < 0:
        return 0.0
    n = np.uint32(np.rint(np.float64(v)))
    return float(np.array(n, dtype=np.uint32).view(np.float32))


def _finish(outs, post):
    """outs: per-core [128, 8] partial blocks (cols 2*bi ACT / 2*bi+1 DVE).
    Returns the final loss (f64 host math)."""
    z = np.zeros((128, 8), dtype=np.float64)
    for o in outs:
        z += np.asarray(o, dtype=np.float64)
    # batch row bi*128 + p  <-  z[p, 2*bi] + z[p, 2*bi+1]
    Z = np.empty(B, dtype=np.float64)
    for bi in range(4):
        Z[bi * 128 : (bi + 1) * 128] = z[:, 2 * bi] + z[:, 2 * bi + 1]
    phi, psl, lab = post["phi"], post["psl"], post["lab"]

    dev = np.empty(B, dtype=np.float64)
    for b in range(B):
        lc = int(lab[b] % CS)
        sbi = min(lc // SUPER, len(SBS) - 1)
        j = lc - sbi * SUPER
        ga = min(GA, SBS[sbi])
        if j < ga:
            dev[b] = math.exp(S * psl[b] / (QS * QS) - M0)
        else:
            dev[b] = _schraudolph_host(psl[b])

    Zp = Z - dev + np.exp(S * phi - M0)
    nll = M0 + np.log(Zp) - S * phi
    return float(np.mean(nll))


def kernel(input, label, weight):
    global _GRAPH, LAST_RESULT
    xt, shards, post = _host_prep(input, label, weight)
    if _GRAPH is None:
        _GRAPH = _build_nc()
    in_maps = [{"xt": xt, "wt": shards[i]} for i in range(NCORES)]
    res = run_bass_kernel_spmd(_GRAPH, in_maps, list(range(NCORES)))
    LAST_RESULT = res
    loss = _finish([res.results[i]["out"] for i in range(NCORES)], post)
    return np.float32(loss).reshape(())


# revision 11
# speedup vs baseline: 1.1318x; 1.1318x over previous
"""ArcFace loss on 8 TRN2 NeuronCores — class-parallel, fp8, hybrid exp.

Math: loss = mean_b[ M0 + ln(Z'_b) - s*phi_b ] with
  Z_b  = sum_c exp(s*cos(b,c) - M0)          (device, sharded over classes)
  Z'_b = Z_b - dev(b, l_b) + exp(s*phi_b - M0)
dev(b, l) is the device's own contribution for the label class (host
replicates it bit-for-bit so the correction cancels); phi uses the exact
f64 cosine. M0 is a fixed logsumexp shift.

Device per core: x and W rows unit-normalized on host, scaled by QS=32,
fp8 e4m3. The 512x512x12544 matmul runs in DoubleRow perf mode. Loop
order keeps each stationary x-tile loaded for 4 consecutive N=512
matmuls (1 LDWEIGHTS per 4 MMs instead of 1:1). All W superblocks are
DMA'd up front and stay SBUF-resident; ~10 warm-up matmuls on a zeroed
scratch run during the DMA head so the PE HAM clock-gate is released
(2.4 GHz) before real work arrives.

exp is split across two engines per 2048-col superblock:
 - ACT: cols [0:GA): exp activation with accum_out (sum comes for free,
   no DVE fold needed).
 - DVE: cols [GA:2048): Schraudolph bit-trick exp2 — one tensor_scalar
   (p*A + B -> uint32, negative inputs saturate to 0 = free relu; RNE
   convert materializes the f32 bit pattern of 2^t) and one accumulating
   tensor_scalar over the bitcast-f32 values. c = 1.5 - 1/ln2 makes the
   value-weighted mean of the piecewise-linear error exactly 1, so no
   host-side correction factor is needed.
Per-core output is [128, 8]: per batch-tile ACT and DVE partial sums.
The cross-core sum, label correction, ln and mean run on the host in
f64 — no device collective.
"""

import math

import numpy as np

from concourse import bacc, mybir
from concourse.bass_utils import run_bass_kernel_spmd
from concourse.tile import TileContext

NCORES = 8
B = 512
D = 512
C = 100000
CS = 12544  # per-core classes, padded: 8 * 12544 = 100352 >= C
S = 120.0
MARGIN = 0.3
COS_M = math.cos(MARGIN)
SIN_M = math.sin(MARGIN)
TH = math.cos(math.pi - MARGIN)
MM = math.sin(math.pi - MARGIN) * MARGIN
M0 = 40.0  # logsumexp shift
QS = 32.0  # fp8 quantization scale for x and W (unit rows -> |elem*QS| <= 32)
SUPER = 2048  # class columns per superblock
NBLK = 512  # class columns per matmul (one PSUM bank)
SBS = [SUPER] * 6 + [256]  # superblock widths; sum == CS
assert sum(SBS) == CS
GA = 1408  # ACT-exp columns per 2048 superblock (DVE gets SUPER-GA)
GD = SUPER - GA
WARM = 16  # PE warm-up matmuls during the DMA head

LOG2E = 1.4426950408889634
C_SCH = 1.5 - 1.0 / math.log(2.0)  # value-weighted-unbiased Schraudolph shift
A_SCH = float(np.float32(S * LOG2E / (QS * QS) * 8388608.0))
B_SCH = float(np.float32((127.0 - C_SCH - M0 * LOG2E) * 8388608.0))

F32 = mybir.dt.float32
U32 = mybir.dt.uint32
BF16 = mybir.dt.bfloat16
F8 = mybir.dt.float8e4
FN = mybir.ActivationFunctionType
DR = mybir.MatmulPerfMode.DoubleRow
MULT = mybir.AluOpType.mult
ADD = mybir.AluOpType.add

_GRAPH = None
LAST_RESULT = None  # BassKernelResults of the most recent run (for test harness)


def _build_nc(repeat=1):
    """Build the SPMD graph. repeat>1 unrolls the whole body N times into one
    NEFF (timing only: amortizes the per-execute dispatch overhead)."""
    nc = bacc.Bacc("TRN2", target_bir_lowering=False)

    # x^T fp8, DoubleRow pairs: row kp*128+p, col i*B+b = x[b, (2kp+i)*128+p]
    xt = nc.declare_dram_parameter("xt", [256, 2 * B], F8, isOutput=False)
    # W^T fp8, DoubleRow pairs, superblock-major: per pair row-block and
    # superblock (c0, sw), cols [2*c0 : 2*c0+2*sw] hold [2, sw] row-major
    wt = nc.declare_dram_parameter("wt", [256, 2 * CS], F8, isOutput=False)
    # per-core partials: col 2*bi = ACT sum, 2*bi+1 = DVE sum, row p =
    # batch bi*128+p
    out = nc.declare_dram_parameter("out", [128, 8], F32, isOutput=True)

    with TileContext(nc, num_cores=NCORES) as tc:
        with (
            tc.tile_pool(name="xpool", bufs=1) as xpool,
            tc.tile_pool(name="wpool", bufs=1) as wpool,
            tc.tile_pool(name="spool", bufs=1) as spool,
            tc.tile_pool(name="zpool", bufs=1) as zpool,
            tc.tile_pool(name="psum", bufs=2, space="PSUM") as pp,
        ):
            # PE warm-up: dependency-free matmuls on scratch SBUF (values
            # discarded; the ps buffer is overwritten by the first real
            # start=True matmul). Keeps PE busy through the DMA head so
            # the HAM clock-gate opens (K=8/8) before real work arrives.
            wsc = xpool.tile([128, 2, NBLK], F8, tag="wsc", name="wsc")
            nc.vector.memset(wsc[:], 0.0)
            # exp bias constant as a tracked tile (avoids a global barrier)
            cb = xpool.tile([128, 1], F32, tag="cb", name="cb")
            nc.vector.memset(cb[:], -M0)
            ps_w = pp.tile([128, SUPER], F32, tag="ps", name="ps_warm")
            for i in range(WARM):
                nc.tensor.matmul(
                    ps_w[:, :NBLK],
                    wsc[:, :, :128],
                    wsc[:],
                    start=True,
                    stop=True,
                    perf_mode=DR,
                )

            # x^T fp8 pair tiles [K=128, sub=2, B]
            xts = []
            for kp in range(2):
                t = xpool.tile([128, 2, B], F8, tag=f"xt{kp}", name=f"xts{kp}")
                nc.sync.dma_start(
                    t[:],
                    xt[kp * 128 : (kp + 1) * 128, :].rearrange(
                        "p (s b) -> p s b", s=2
                    ),
                )
                xts.append(t)

            for rep in range(repeat):
                _body(nc, tc, rep, wpool, spool, zpool, pp, wt, out, xts, cb)

    if not nc.is_finalized():
        nc.finalize()
    return nc


def _body(nc, tc, rep, wpool, spool, zpool, pp, wt, out, xts, cb):
    # all W superblocks up front, SBUF-resident for the whole rep
    wts = []
    c0 = 0
    for sbi, sw in enumerate(SBS):
        pair = []
        for kp in range(2):
            t = wpool.tile(
                [128, 2, sw], F8, tag=f"w{sbi}_{kp}", name=f"w{sbi}_{kp}_{rep}"
            )
            nc.sync.dma_start(
                t[:],
                wt[
                    kp * 128 : (kp + 1) * 128, 2 * c0 : 2 * c0 + 2 * sw
                ].rearrange("p (s c) -> p s c", s=2),
            )
            pair.append(t)
        wts.append(pair)
        c0 += sw

    # per-batch-tile partial sums, one col per superblock (ACT / DVE split)
    zbA = [
        zpool.tile([128, 8], F32, tag=f"za{bi}", name=f"za{bi}_{rep}")
        for bi in range(4)
    ]
    zbD = [
        zpool.tile([128, 8], F32, tag=f"zd{bi}", name=f"zd{bi}_{rep}")
        for bi in range(4)
    ]
    # scratch outputs (values unused; accum_out carries the result)
    exs = spool.tile([128, GA], BF16, tag="exs", name=f"exs_{rep}")
    sus = spool.tile([128, GD], U32, tag="sus", name=f"sus_{rep}")
    su2 = spool.tile([128, GD], BF16, tag="su2", name=f"su2_{rep}")

    zs_all = zpool.tile([128, 8], F32, tag="zsall", name=f"zsall_{rep}")
    last_sb = len(SBS) - 1
    for sbi, sw in enumerate(SBS):
        ga = min(GA, sw)
        for bi in range(4):
            ps = pp.tile([128, SUPER], F32, tag="ps", name=f"ps_{rep}")
            for kp in range(2):
                for nb0 in range(0, sw, NBLK):
                    nb = min(NBLK, sw - nb0)
                    nc.tensor.matmul(
                        ps[:, nb0 : nb0 + nb],
                        xts[kp][:, :, bi * 128 : (bi + 1) * 128],
                        wts[sbi][kp][:, :, nb0 : nb0 + nb],
                        start=(kp == 0),
                        stop=(kp == 1),
                        perf_mode=DR,
                    )
            nc.scalar.activation(
                exs[:, :ga],
                ps[:, :ga],
                FN.Exp,
                bias=cb[:, 0:1],
                scale=S / (QS * QS),
                accum_out=zbA[bi][:, sbi : sbi + 1],
            )
            if sw > ga:
                gd = sw - ga
                nc.vector.tensor_scalar(
                    sus[:, :gd], ps[:, ga:sw], A_SCH, B_SCH, MULT, ADD
                )
                nc.vector.tensor_scalar(
                    su2[:, :gd],
                    sus[:, :gd].bitcast(F32),
                    1.0,
                    0.0,
                    MULT,
                    ADD,
                    accum_out=zbD[bi][:, sbi : sbi + 1],
                )
            if sbi == last_sb:
                # this bi is complete: fold its per-superblock partials and
                # ship them out while later bi tiles still compute.
                # zbA cols are ACT accumulator outputs; a cross-engine read
                # straight off an accum_out races the accumulator drain
                # (observed flaky stale/NaN reads), so ACT itself stages a
                # copy — same-queue FIFO makes that ordering airtight — and
                # the DVE reduce reads the staged copy.
                zc = zpool.tile([128, 8], F32, tag=f"zc{bi}",
                                name=f"zc{bi}_{rep}")
                nc.scalar.copy(zc[:, : len(SBS)], zbA[bi][:, : len(SBS)])
                nc.vector.reduce_sum(
                    zs_all[:, 2 * bi : 2 * bi + 1],
                    zc[:, : len(SBS)],
                    axis=mybir.AxisListType.X,
                )
                nc.vector.reduce_sum(
                    zs_all[:, 2 * bi + 1 : 2 * bi + 2],
                    zbD[bi][:, : len(SBS) - 1],
                    axis=mybir.AxisListType.X,
                )
                nc.sync.dma_start(
                    out[:, 2 * bi : 2 * bi + 2],
                    zs_all[:, 2 * bi : 2 * bi + 2],
                )


def _dr_pack(aT):
    """[D, N] (D=512) -> [256, 2*N]: DoubleRow pair layout. Row kp*128+p,
    col i*N+n = aT[(2*kp+i)*128 + p, n]."""
    d, n = aT.shape
    chunks = aT.reshape(4, 128, n)
    pairs = [
        np.stack([chunks[2 * kp], chunks[2 * kp + 1]], axis=1).reshape(
            128, 2 * n
        )
        for kp in range(2)
    ]
    return np.concatenate(pairs, axis=0)


def _host_prep(input, label, weight):
    x = np.asarray(input, dtype=np.float32)
    lab = np.asarray(label).astype(np.int64).ravel()
    w = np.asarray(weight, dtype=np.float32)
    f8 = mybir.dt.np(F8)

    xn64 = x.astype(np.float64)
    xn64 /= np.maximum(
        np.sqrt(np.einsum("bd,bd->b", xn64, xn64))[:, None], 1e-12
    )
    xq = (xn64 * QS).astype(np.float32).astype(f8)  # [B, D] fp8
    xt = np.ascontiguousarray(_dr_pack(xq.astype(np.float32).T).astype(f8))

    wn_inv = 1.0 / np.maximum(
        np.sqrt(np.einsum("cd,cd->c", w, w, dtype=np.float64)), 1e-12
    )
    wn = w * wn_inv[:, None].astype(np.float32)  # [C, D] normalized rows, f32
    wq = (wn * QS).astype(f8)  # [C, D] fp8

    # label terms (tiny): phi from the exact f64 cosine, the Z-correction
    # from the fp8 cosine the device actually summed
    wl = wn[lab].astype(np.float64)  # [B, D]
    cosl = np.einsum("bd,bd->b", xn64, wl)
    cosl = np.clip(cosl, -1.0, 1.0)
    sine = np.sqrt(np.maximum(1.0 - cosl * cosl, 0.0))
    phi = cosl * COS_M - sine * SIN_M
    phi = np.where(cosl > TH, phi, cosl - MM)
    psl = np.einsum(
        "bd,bd->b",
        xq.astype(np.float32),
        wq[lab].astype(np.float32),
        dtype=np.float64,
    )  # device psum value for the label column (= cosq * QS^2)
    post = {"phi": phi, "psl": psl, "lab": lab}

    # class-sharded, transposed, DoubleRow-packed, superblock-major W
    shards = []
    for i in range(NCORES):
        lo, hi = i * CS, min((i + 1) * CS, C)
        sh = np.zeros((CS, D), dtype=f8)
        sh[: hi - lo] = wq[lo:hi]
        packed = _dr_pack(sh.astype(np.float32).T)  # [256, 2*CS], pair layout
        # rearrange cols to superblock-major [2, sw] blocks
        dst = np.empty_like(packed)
        q = 0
        c0 = 0
        for sw in SBS:
            blk = packed.reshape(256, 2, CS)[:, :, c0 : c0 + sw]
            dst[:, q : q + 2 * sw] = blk.reshape(256, 2 * sw)
            q += 2 * sw
            c0 += sw
        shards.append(np.ascontiguousarray(dst.astype(f8)))
    return xt, shards, post


def _schraudolph_host(p):
    """Replicate the device DVE bit-trick for a scalar psum value p (f64)."""
    v = np.float32(np.float32(np.float32(p) * np.float32(A_SCH)) +
                   np.float32(B_SCH))
    if v < 0:
        return 0.0
    n = np.uint32(np.rint(np.float64(v)))
    return float(np.array(n, dtype=np.uint32).view(np.float32))


def _finish(outs, post):
    """outs: per-core [128, 8] partial blocks (cols 2*bi ACT / 2*bi+1 DVE).
    Returns the final loss (f64 host math)."""
    z = np.zeros((128, 8), dtype=np.float64)
    for o in outs:
        z += np.asarray(o, dtype=np.float64)
    # batch row bi*128 + p  <-  z[p, 2*bi] + z[p, 2*bi+1]
    Z = np.empty(B, dtype=np.float64)
    for bi in range(4):
        Z[bi * 128 : (bi + 1) * 128] = z[:, 2 * bi] + z[:, 2 * bi + 1]
    phi, psl, lab = post["phi"], post["psl"], post["lab"]

    dev = np.empty(B, dtype=np.float64)
    for b in range(B):
        lc = int(lab[b] % CS)
        sbi = min(lc // SUPER, len(SBS) - 1)
        j = lc - sbi * SUPER
        ga = min(GA, SBS[sbi])
        if j < ga:
            dev[b] = math.exp(S * psl[b] / (QS * QS) - M0)
        else:
            dev[b] = _schraudolph_host(psl[b])

    Zp = Z - dev + np.exp(S * phi - M0)
    nll = M0 + np.log(Zp) - S * phi
    return float(np.mean(nll))


def kernel(input, label, weight):
    global _GRAPH, LAST_RESULT
    xt, shards, post = _host_prep(input, label, weight)
    if _GRAPH is None:
        _GRAPH = _build_nc()
    in_maps = [{"xt": xt, "wt": shards[i]} for i in range(NCORES)]
    res = run_bass_kernel_spmd(_GRAPH, in_maps, list(range(NCORES)))
    LAST_RESULT = res
    loss = _finish([res.results[i]["out"] for i in range(NCORES)], post)
    return np.float32(loss).reshape(())


# revision 17
# speedup vs baseline: 1.2547x; 1.1086x over previous
"""ArcFace loss on 8 TRN2 NeuronCores — class-parallel, fp8, hybrid exp.

Math: loss = mean_b[ M0 + ln(Z'_b) - s*phi_b ] with
  Z_b  = sum_c exp(s*cos(b,c) - M0)          (device, sharded over classes)
  Z'_b = Z_b - dev(b, l_b) + exp(s*phi_b - M0)
dev(b, l) is the device's own contribution for the label class (host
replicates it bit-for-bit so the correction cancels); phi uses the exact
f64 cosine. M0 is a fixed logsumexp shift.

Device per core: x and W rows unit-normalized on host, scaled by QS=32,
fp8 e4m3. The 512x512x12544 matmul runs in DoubleRow perf mode. Loop
order keeps each stationary x-tile loaded for 4 consecutive N=512
matmuls (1 LDWEIGHTS per 4 MMs instead of 1:1). All W superblocks are
DMA'd up front and stay SBUF-resident; ~10 warm-up matmuls on a zeroed
scratch run during the DMA head so the PE HAM clock-gate is released
(2.4 GHz) before real work arrives.

exp is split across two engines per 2048-col superblock:
 - ACT: cols [0:GA): exp activation with accum_out (sum comes for free,
   no DVE fold needed).
 - DVE: cols [GA:2048): Schraudolph bit-trick exp2 — one tensor_scalar
   (p*A + B -> uint32, negative inputs saturate to 0 = free relu; RNE
   convert materializes the f32 bit pattern of 2^t) and one accumulating
   tensor_scalar over the bitcast-f32 values. c = 1.5 - 1/ln2 makes the
   value-weighted mean of the piecewise-linear error exactly 1, so no
   host-side correction factor is needed.
Per-core output is [128, 8]: per batch-tile ACT and DVE partial sums.
The cross-core sum, label correction, ln and mean run on the host in
f64 — no device collective.
"""

import math

import numpy as np

from concourse import bacc, mybir
from concourse.bass_utils import run_bass_kernel_spmd
from concourse.tile import TileContext

NCORES = 8
B = 512
D = 512
C = 100000
CS = 12544  # per-core classes, padded: 8 * 12544 = 100352 >= C
S = 120.0
MARGIN = 0.3
COS_M = math.cos(MARGIN)
SIN_M = math.sin(MARGIN)
TH = math.cos(math.pi - MARGIN)
MM = math.sin(math.pi - MARGIN) * MARGIN
M0 = 40.0  # logsumexp shift
QS = 32.0  # fp8 quantization scale for x and W (unit rows -> |elem*QS| <= 32)
SUPER = 2048  # class columns per superblock
NBLK = 512  # class columns per matmul (one PSUM bank)
SBS = [256] + [SUPER] * 6  # superblock widths; sum == CS. The short tail
# superblock runs FIRST so the final quantum's consumer chain is short.
assert sum(SBS) == CS
GA = 1536  # ACT-exp columns per 2048 superblock: its own 3-bank PSUM tile
GD = SUPER - GA  # DVE columns: separate 1-bank PSUM tile
WARM = 16  # PE warm-up matmuls during the DMA head

LOG2E = 1.4426950408889634
C_SCH = 1.5 - 1.0 / math.log(2.0)  # value-weighted-unbiased Schraudolph shift
A_SCH = float(np.float32(S * LOG2E / (QS * QS) * 8388608.0))
B_SCH = float(np.float32((127.0 - C_SCH - M0 * LOG2E) * 8388608.0))

F32 = mybir.dt.float32
U32 = mybir.dt.uint32
BF16 = mybir.dt.bfloat16
F8 = mybir.dt.float8e4
FN = mybir.ActivationFunctionType
DR = mybir.MatmulPerfMode.DoubleRow
MULT = mybir.AluOpType.mult
ADD = mybir.AluOpType.add

_GRAPH = None
LAST_RESULT = None  # BassKernelResults of the most recent run (for test harness)


def _build_nc(repeat=1):
    """Build the SPMD graph. repeat>1 unrolls the whole body N times into one
    NEFF (timing only: amortizes the per-execute dispatch overhead)."""
    nc = bacc.Bacc("TRN2", target_bir_lowering=False)

    # x^T fp8, DoubleRow pairs: row kp*128+p, col i*B+b = x[b, (2kp+i)*128+p]
    xt = nc.declare_dram_parameter("xt", [256, 2 * B], F8, isOutput=False)
    # W^T fp8, DoubleRow pairs, superblock-major: per pair row-block and
    # superblock (c0, sw), cols [2*c0 : 2*c0+2*sw] hold [2, sw] row-major
    wt = nc.declare_dram_parameter("wt", [256, 2 * CS], F8, isOutput=False)
    # per-core partials: col 2*bi = ACT sum, 2*bi+1 = DVE sum, row p =
    # batch bi*128+p
    out = nc.declare_dram_parameter("out", [128, 8], F32, isOutput=True)

    with TileContext(nc, num_cores=NCORES) as tc:
        with (
            tc.tile_pool(name="xpool", bufs=1) as xpool,
            tc.tile_pool(name="wpool", bufs=1) as wpool,
            tc.tile_pool(name="spool", bufs=1) as spool,
            tc.tile_pool(name="zpool", bufs=1) as zpool,
            tc.tile_pool(name="psum", bufs=2, space="PSUM") as pp,
        ):
            # PE warm-up: matmuls on a small zeroed scratch, values discarded
            # (the ps buffer is overwritten by the first real start=True
            # matmul). Keeps PE busy through the preamble + DMA head so the
            # HAM clock-gate opens (K=8/8) before real work arrives. N=64
            # keeps the scratch memset (the only dependency) cheap.
            wsc = xpool.tile([128, 2, 192], F8, tag="wsc", name="wsc")
            nc.vector.memset(wsc[:], 0.0)
            # exp bias constant as a tracked tile (avoids a global barrier)
            cb = xpool.tile([128, 1], F32, tag="cb", name="cb")
            nc.vector.memset(cb[:], -M0)
            ps_w = pp.tile([128, GA], F32, tag="pa", name="ps_warm")
            for i in range(WARM):
                nc.tensor.matmul(
                    ps_w[:, :64],
                    wsc[:, :, :128],
                    wsc[:, :, 128:192],
                    start=True,
                    stop=True,
                    perf_mode=DR,
                )

            # x^T fp8 pair tiles [K=128, sub=2, B]
            xts = []
            for kp in range(2):
                t = xpool.tile([128, 2, B], F8, tag=f"xt{kp}", name=f"xts{kp}")
                nc.sync.dma_start(
                    t[:],
                    xt[kp * 128 : (kp + 1) * 128, :].rearrange(
                        "p (s b) -> p s b", s=2
                    ),
                )
                xts.append(t)

            for rep in range(repeat):
                _body(nc, tc, rep, wpool, spool, zpool, pp, wt, out, xts, cb)

    if not nc.is_finalized():
        nc.finalize()
    return nc


def _body(nc, tc, rep, wpool, spool, zpool, pp, wt, out, xts, cb):
    # all W superblocks up front, SBUF-resident for the whole rep
    wts = []
    c0 = 0
    for sbi, sw in enumerate(SBS):
        pair = []
        for kp in range(2):
            t = wpool.tile(
                [128, 2, sw], F8, tag=f"w{sbi}_{kp}", name=f"w{sbi}_{kp}_{rep}"
            )
            nc.sync.dma_start(
                t[:],
                wt[
                    kp * 128 : (kp + 1) * 128, 2 * c0 : 2 * c0 + 2 * sw
                ].rearrange("p (s c) -> p s c", s=2),
            )
            pair.append(t)
        wts.append(pair)
        c0 += sw

    # per-batch-tile partial sums, one col per superblock (ACT / DVE split)
    zbA = [
        zpool.tile([128, 8], F32, tag=f"za{bi}", name=f"za{bi}_{rep}")
        for bi in range(4)
    ]
    zbD = [
        zpool.tile([128, 8], F32, tag=f"zd{bi}", name=f"zd{bi}_{rep}")
        for bi in range(4)
    ]
    # scratch outputs (values unused; accum_out carries the result)
    exs = spool.tile([128, GA], BF16, tag="exs", name=f"exs_{rep}")
    sus = spool.tile([128, GD], U32, tag="sus", name=f"sus_{rep}")
    su2 = spool.tile([128, GD], BF16, tag="su2", name=f"su2_{rep}")

    zs_all = zpool.tile([128, 8], F32, tag="zsall", name=f"zsall_{rep}")
    last_sb = len(SBS) - 1
    for sbi, sw in enumerate(SBS):
        ga = min(GA, sw)
        for bi in range(4):
            # ACT and DVE columns live in separate PSUM tiles so each
            # engine's completion frees its own buffer independently
            pa = pp.tile([128, GA], F32, tag="pa", name=f"pa_{rep}")
            pd = (
                pp.tile([128, GD], F32, tag="pd", name=f"pd_{rep}")
                if sw > ga
                else None
            )
            for kp in range(2):
                for nb0 in range(0, sw, NBLK):
                    nb = min(NBLK, sw - nb0)
                    dst = (
                        pa[:, nb0 : nb0 + nb]
                        if nb0 < ga
                        else pd[:, nb0 - ga : nb0 - ga + nb]
                    )
                    nc.tensor.matmul(
                        dst,
                        xts[kp][:, :, bi * 128 : (bi + 1) * 128],
                        wts[sbi][kp][:, :, nb0 : nb0 + nb],
                        start=(kp == 0),
                        stop=(kp == 1),
                        perf_mode=DR,
                    )
            nc.scalar.activation(
                exs[:, :ga],
                pa[:, :ga],
                FN.Exp,
                bias=cb[:, 0:1],
                scale=S / (QS * QS),
                accum_out=zbA[bi][:, sbi : sbi + 1],
            )
            if pd is not None:
                nc.vector.tensor_scalar(
                    sus[:], pd[:], A_SCH, B_SCH, MULT, ADD
                )
                nc.vector.tensor_scalar(
                    su2[:],
                    sus[:].bitcast(F32),
                    1.0,
                    0.0,
                    MULT,
                    ADD,
                    accum_out=zbD[bi][:, sbi : sbi + 1],
                )
            if sbi == last_sb:
                # this bi is complete: fold its per-superblock partials and
                # ship them out while later bi tiles still compute.
                # zbA cols are ACT accumulator outputs; a cross-engine read
                # straight off an accum_out races the accumulator drain
                # (observed flaky stale/NaN reads), so ACT itself stages a
                # copy — same-queue FIFO makes that ordering airtight — and
                # the DVE reduce reads the staged copy.
                zc = zpool.tile([128, 8], F32, tag=f"zc{bi}",
                                name=f"zc{bi}_{rep}")
                nc.scalar.copy(zc[:, : len(SBS)], zbA[bi][:, : len(SBS)])
                nc.vector.reduce_sum(
                    zs_all[:, 2 * bi : 2 * bi + 1],
                    zc[:, : len(SBS)],
                    axis=mybir.AxisListType.X,
                )
                # DVE partials exist only for the full 2048 superblocks
                # (sbi 1..6 — sb 0 is the 256-col ACT-only tail)
                nc.vector.reduce_sum(
                    zs_all[:, 2 * bi + 1 : 2 * bi + 2],
                    zbD[bi][:, 1 : len(SBS)],
                    axis=mybir.AxisListType.X,
                )
                nc.sync.dma_start(
                    out[:, 2 * bi : 2 * bi + 2],
                    zs_all[:, 2 * bi : 2 * bi + 2],
                )


def _dr_pack(aT):
    """[D, N] (D=512) -> [256, 2*N]: DoubleRow pair layout. Row kp*128+p,
    col i*N+n = aT[(2*kp+i)*128 + p, n]."""
    d, n = aT.shape
    chunks = aT.reshape(4, 128, n)
    pairs = [
        np.stack([chunks[2 * kp], chunks[2 * kp + 1]], axis=1).reshape(
            128, 2 * n
        )
        for kp in range(2)
    ]
    return np.concatenate(pairs, axis=0)


def _host_prep(input, label, weight):
    x = np.asarray(input, dtype=np.float32)
    lab = np.asarray(label).astype(np.int64).ravel()
    w = np.asarray(weight, dtype=np.float32)
    f8 = mybir.dt.np(F8)

    xn64 = x.astype(np.float64)
    xn64 /= np.maximum(
        np.sqrt(np.einsum("bd,bd->b", xn64, xn64))[:, None], 1e-12
    )
    xq = (xn64 * QS).astype(np.float32).astype(f8)  # [B, D] fp8
    xt = np.ascontiguousarray(_dr_pack(xq.astype(np.float32).T).astype(f8))

    wn_inv = 1.0 / np.maximum(
        np.sqrt(np.einsum("cd,cd->c", w, w, dtype=np.float64)), 1e-12
    )
    wn = w * wn_inv[:, None].astype(np.float32)  # [C, D] normalized rows, f32
    wq = (wn * QS).astype(f8)  # [C, D] fp8

    # label terms (tiny): phi from the exact f64 cosine, the Z-correction
    # from the fp8 cosine the device actually summed
    wl = wn[lab].astype(np.float64)  # [B, D]
    cosl = np.einsum("bd,bd->b", xn64, wl)
    cosl = np.clip(cosl, -1.0, 1.0)
    sine = np.sqrt(np.maximum(1.0 - cosl * cosl, 0.0))
    phi = cosl * COS_M - sine * SIN_M
    phi = np.where(cosl > TH, phi, cosl - MM)
    psl = np.einsum(
        "bd,bd->b",
        xq.astype(np.float32),
        wq[lab].astype(np.float32),
        dtype=np.float64,
    )  # device psum value for the label column (= cosq * QS^2)
    post = {"phi": phi, "psl": psl, "lab": lab}

    # class-sharded, transposed, DoubleRow-packed, superblock-major W
    shards = []
    for i in range(NCORES):
        lo, hi = i * CS, min((i + 1) * CS, C)
        sh = np.zeros((CS, D), dtype=f8)
        sh[: hi - lo] = wq[lo:hi]
        packed = _dr_pack(sh.astype(np.float32).T)  # [256, 2*CS], pair layout
        # rearrange cols to superblock-major [2, sw] blocks
        dst = np.empty_like(packed)
        q = 0
        c0 = 0
        for sw in SBS:
            blk = packed.reshape(256, 2, CS)[:, :, c0 : c0 + sw]
            dst[:, q : q + 2 * sw] = blk.reshape(256, 2 * sw)
            q += 2 * sw
            c0 += sw
        shards.append(np.ascontiguousarray(dst.astype(f8)))
    return xt, shards, post


def _schraudolph_host(p):
    """Replicate the device DVE bit-trick for a scalar psum value p (f64)."""
    v = np.float32(np.float32(np.float32(p) * np.float32(A_SCH)) +
                   np.float32(B_SCH))
    if v < 0:
        return 0.0
    n = np.uint32(np.rint(np.float64(v)))
    return float(np.array(n, dtype=np.uint32).view(np.float32))


def _finish(outs, post):
    """outs: per-core [128, 8] partial blocks (cols 2*bi ACT / 2*bi+1 DVE).
    Returns the final loss (f64 host math)."""
    z = np.zeros((128, 8), dtype=np.float64)
    for o in outs:
        z += np.asarray(o, dtype=np.float64)
    # batch row bi*128 + p  <-  z[p, 2*bi] + z[p, 2*bi+1]
    Z = np.empty(B, dtype=np.float64)
    for bi in range(4):
        Z[bi * 128 : (bi + 1) * 128] = z[:, 2 * bi] + z[:, 2 * bi + 1]
    phi, psl, lab = post["phi"], post["psl"], post["lab"]

    dev = np.empty(B, dtype=np.float64)
    for b in range(B):
        lc = int(lab[b] % CS)
        c0 = 0
        for sw in SBS:
            if lc < c0 + sw:
                break
            c0 += sw
        j = lc - c0
        if j < min(GA, sw):
            dev[b] = math.exp(S * psl[b] / (QS * QS) - M0)
        else:
            dev[b] = _schraudolph_host(psl[b])

    Zp = Z - dev + np.exp(S * phi - M0)
    nll = M0 + np.log(Zp) - S * phi
    return float(np.mean(nll))


def kernel(input, label, weight):
    global _GRAPH, LAST_RESULT
    xt, shards, post = _host_prep(input, label, weight)
    if _GRAPH is None:
        _GRAPH = _build_nc()
    in_maps = [{"xt": xt, "wt": shards[i]} for i in range(NCORES)]
    res = run_bass_kernel_spmd(_GRAPH, in_maps, list(range(NCORES)))
    LAST_RESULT = res
    loss = _finish([res.results[i]["out"] for i in range(NCORES)], post)
    return np.float32(loss).reshape(())
